# revision 15
# baseline (speedup 1.0000x reference)
# Trainium2 Bass kernel for nn_DinoDecoderBlock (B=8, NQ=NK=1024, C=768, H=12).
#
# Sharding: data-parallel over batch — batch element b runs on core b (8 cores,
# no collectives). Each core computes the full decoder block for its element.
#
# Device layout: every activation is feature-major [C, Ntok] so all GEMMs keep
# the contraction dim on SBUF partitions (weights stationary, activations
# moving) and compose with zero on-device transposes. V is produced
# token-major by swapping stationary/moving roles. Softmax denominators come
# free from a ones-column appended to V (M=65 matmul); 1/Z is applied to the
# small O tile via a K=1 broadcast matmul. LayerNorm gamma is folded into the
# consuming weights host-side; LN beta folds into downstream eviction biases.
# Softmax max-subtraction is skipped: scores are bounded (|s| < ~3) by
# construction, so exp is safe in fp32.
import numpy as np

B, NQ, NK, C, H = 8, 1024, 1024, 768, 12
HD = C // H          # 64
HID = 4 * C          # 3072
EPS = 1e-5
SCALE = HD ** -0.5
P = 128
FD = 512
KC = C // P          # 6 feature chunks
NKC = NK // P        # 8 key-token chunks
NQT = NQ // FD       # 2 query tiles
MQK = 2 * C // P     # 12 output chunks for fused Q,K
MH = HID // P        # 24

_CACHE = {}


def _np_dt(dt):
    from concourse import mybir
    return np.dtype(mybir.dt.np(dt))


def _prep(inputs):
    """Host-side prep: fold LN gamma/beta into weights/biases, transpose to
    feature-major, cast matmul weights to bf16."""
    from concourse import mybir
    bf16 = _np_dt(mybir.dt.bfloat16)
    f32 = np.float32

    g1 = np.asarray(inputs["ln1_g"], f32); b1 = np.asarray(inputs["ln1_b"], f32)
    g2 = np.asarray(inputs["ln2_g"], f32); b2 = np.asarray(inputs["ln2_b"], f32)
    g3 = np.asarray(inputs["ln3_g"], f32); b3 = np.asarray(inputs["ln3_b"], f32)
    gy = np.asarray(inputs["lny_g"], f32); by = np.asarray(inputs["lny_b"], f32)

    qkv_w = np.asarray(inputs["qkv_w"], f32)          # [2304, 768]
    wqk = qkv_w[: 2 * C] * g1[None, :]                # [1536, 768]
    wv = qkv_w[2 * C:] * g1[None, :]                  # [768, 768]
    bqk = qkv_w[: 2 * C] @ b1                         # [1536]
    bv = qkv_w[2 * C:] @ b1                           # [768]

    q_w = np.asarray(inputs["q_w"], f32)
    k_w = np.asarray(inputs["k_w"], f32)
    v_w = np.asarray(inputs["v_w"], f32)
    wq2 = q_w * g2[None, :]; bq2 = q_w @ b2
    wk2 = k_w * gy[None, :]; bk2 = k_w @ by
    wv2 = v_w * gy[None, :]; bv2 = v_w @ by

    fc1_w = np.asarray(inputs["fc1_w"], f32)
    wfc1 = fc1_w * g3[None, :]
    bfc1 = np.asarray(inputs["fc1_b"], f32) + fc1_w @ b3

    def as_bias_pm(vec):
        # [M*128] -> [128, M]: column m = per-partition bias of output chunk m
        v = np.asarray(vec, f32)
        return np.ascontiguousarray(v.reshape(-1, P).T)

    maskT = np.ascontiguousarray(
        np.asarray(inputs["mask"]).astype(f32).T).astype(bf16)  # [NK, NQ]

    shared = {
        "wqkT": np.ascontiguousarray(wqk.T).astype(bf16),           # [768,1536]
        "wvT": np.ascontiguousarray(wv.T).astype(bf16),             # [768,768]
        "wprojT": np.ascontiguousarray(
            np.asarray(inputs["attn_proj_w"], f32).T).astype(bf16),
        "wq2T": np.ascontiguousarray(wq2.T).astype(bf16),
        "wk2T": np.ascontiguousarray(wk2.T).astype(bf16),
        "wv2T": np.ascontiguousarray(wv2.T).astype(bf16),
        "wcaT": np.ascontiguousarray(
            np.asarray(inputs["ca_proj_w"], f32).T).astype(bf16),
        "wfc1T": np.ascontiguousarray(wfc1.T).astype(bf16),         # [768,3072]
        "wfc2T": np.ascontiguousarray(
            np.asarray(inputs["fc2_w"], f32).T).astype(bf16),       # [3072,768]
        "bqk": as_bias_pm(bqk),                                     # [128,12]
        "bq2": as_bias_pm(bq2),                                     # [128,6]
        "bk2": as_bias_pm(bk2),
        "bproj": as_bias_pm(np.asarray(inputs["attn_proj_b"], f32)),
        "bca": as_bias_pm(np.asarray(inputs["ca_proj_b"], f32)),
        "bfc1": as_bias_pm(bfc1),                                   # [128,24]
        "bfc2": as_bias_pm(np.asarray(inputs["fc2_b"], f32)),
        "maskT": maskT,
        "bv": np.ascontiguousarray(bv.reshape(1, C)),               # [1,768]
        "bv2": np.ascontiguousarray(bv2.reshape(1, C)),
    }
    flags = {
        "use_bv": bool(np.any(bv != 0.0)),
        "use_bv2": bool(np.any(bv2 != 0.0)),
    }

    x = np.asarray(inputs["x"], f32)
    y = np.asarray(inputs["y"], f32)
    per_core = [{"xT": np.ascontiguousarray(x[b].T),
                 "yT": np.ascontiguousarray(y[b].T)} for b in range(B)]
    return shared, per_core, flags


def _emit(ctx, tc, nc, flags):
    from concourse import mybir

    f32 = mybir.dt.float32
    bf16 = mybir.dt.bfloat16
    AF = mybir.ActivationFunctionType
    OP = mybir.AluOpType

    # ---- DRAM I/O ----
    def din(name, shape, dt):
        return nc.dram_tensor(name, shape, dt, kind="ExternalInput").ap()

    xT = din("xT", [C, NQ], f32)
    yT = din("yT", [C, NK], f32)
    maskT = din("maskT", [NK, NQ], bf16)
    wqkT = din("wqkT", [C, 2 * C], bf16)
    wvT = din("wvT", [C, C], bf16)
    wprojT = din("wprojT", [C, C], bf16)
    wq2T = din("wq2T", [C, C], bf16)
    wk2T = din("wk2T", [C, C], bf16)
    wv2T = din("wv2T", [C, C], bf16)
    wcaT = din("wcaT", [C, C], bf16)
    wfc1T = din("wfc1T", [C, HID], bf16)
    wfc2T = din("wfc2T", [HID, C], bf16)
    bqk_d = din("bqk", [P, MQK], f32)
    bq2_d = din("bq2", [P, KC], f32)
    bk2_d = din("bk2", [P, KC], f32)
    bproj_d = din("bproj", [P, KC], f32)
    bca_d = din("bca", [P, KC], f32)
    bfc1_d = din("bfc1", [P, MH], f32)
    bfc2_d = din("bfc2", [P, KC], f32)
    bv_d = din("bv", [1, C], f32)
    bv2_d = din("bv2", [1, C], f32)
    xoutT = nc.dram_tensor("xoutT", [C, NQ], f32, kind="ExternalOutput").ap()

    def chunked(dram_ap, p=P):
        return dram_ap.rearrange("(kc p) m -> p kc m", p=p)

    # ---- long-lived pools ----
    const = ctx.enter_context(tc.tile_pool(name="const", bufs=1))
    masters = ctx.enter_context(tc.tile_pool(name="masters", bufs=2))
    stats = ctx.enter_context(tc.tile_pool(name="stats", bufs=2))
    probs_pool = ctx.enter_context(tc.tile_pool(name="probs", bufs=4))
    wstream = ctx.enter_context(tc.tile_pool(name="wstream", bufs=2))
    psum = ctx.enter_context(tc.tile_pool(name="psum", bufs=2, space="PSUM"))

    # ---- constants ----
    ones_col = const.tile([P, 1], bf16)   # stationary ones: partition sums
    nc.vector.memset(ones_col, 1.0)
    ones_blk = const.tile([P, P], bf16)   # K=1 stationary rows at any base
    nc.vector.memset(ones_blk, 1.0)
    eps_t = const.tile([1, 1], f32)
    nc.vector.memset(eps_t, EPS)

    def load_const(ap_dram, shape, cname):
        t = const.tile(shape, f32, name=cname, tag=cname)
        nc.sync.dma_start(out=t, in_=ap_dram)
        return t
    bqk_t = load_const(bqk_d, [P, MQK], "c_bqk")
    bq2_t = load_const(bq2_d, [P, KC], "c_bq2")
    bk2_t = load_const(bk2_d, [P, KC], "c_bk2")
    bproj_t = load_const(bproj_d, [P, KC], "c_bproj")
    bca_t = load_const(bca_d, [P, KC], "c_bca")
    bfc1_t = load_const(bfc1_d, [P, MH], "c_bfc1")
    bfc2_t = load_const(bfc2_d, [P, KC], "c_bfc2")
    bv_t = load_const(bv_d, [1, C], "c_bv") if flags["use_bv"] else None
    bv2_t = load_const(bv2_d, [1, C], "c_bv2") if flags["use_bv2"] else None

    def stream_w(dram_view):
        """Load a [P, KC_any, <=768] weight block into the shared wmov tag."""
        t = wstream.tile([P, KC, C], bf16, tag="wmov")
        kk, mm = dram_view.shape[1], dram_view.shape[2]
        nc.sync.dma_start(out=t[:, :kk, :mm], in_=dram_view)
        return t

    # ---------------- LayerNorm (feature-major) ----------------
    def layernorm(get_chunk, tmp_pool, out_pool, ntok=NQ, tag="xn"):
        """get_chunk(kc) -> [128, ntok] f32 SBUF AP; called twice per chunk
        (stats pass and normalize pass — streamed sources re-DMA).
        Returns [128, KC, ntok] bf16 tile: (x - mean) * rsqrt(var + eps)."""
        nt = ntok // FD
        sum_x = psum.tile([P, ntok], f32, tag="big")
        sum_sq = psum.tile([P, ntok], f32, tag="big")
        for kc in range(KC):
            src = get_chunk(kc)
            xb = tmp_pool.tile([P, ntok], bf16, tag="xb", bufs=2)
            nc.gpsimd.tensor_copy(xb, src)
            sq = tmp_pool.tile([P, ntok], bf16, tag="sqb", bufs=2)
            nc.gpsimd.tensor_mul(sq, src, src)
            for t in range(nt):
                sl = slice(t * FD, (t + 1) * FD)
                nc.tensor.matmul(sum_x[0:1, sl], ones_col, xb[:, sl],
                                 start=(kc == 0), stop=(kc == KC - 1))
                nc.tensor.matmul(sum_sq[0:1, sl], ones_col, sq[:, sl],
                                 start=(kc == 0), stop=(kc == KC - 1))
        mean = stats.tile([1, ntok], f32, tag="st_mean", bufs=1)
        nc.vector.tensor_scalar_mul(mean, sum_x[0:1, :], 1.0 / C)
        msq = stats.tile([1, ntok], f32, tag="st_msq", bufs=1)
        nc.vector.tensor_scalar_mul(msq, sum_sq[0:1, :], 1.0 / C)
        scratch = stats.tile([1, ntok], f32, tag="st_scratch", bufs=1)
        nc.vector.tensor_tensor(scratch, mean, mean, OP.mult)
        nc.vector.tensor_tensor(msq, msq, scratch, OP.subtract)  # var
        nc.scalar.activation(scratch, msq, AF.Sqrt, bias=eps_t)  # sd
        rstd = stats.tile([1, ntok], f32, tag="st_rstd", bufs=1)
        nc.vector.reciprocal_approx_fast(rstd, scratch)
        meanb = stats.tile([1, ntok], bf16, tag="st_meanb", bufs=1)
        nc.vector.tensor_copy(meanb, mean)
        rstdb = stats.tile([1, ntok], bf16, tag="st_rstdb", bufs=1)
        nc.vector.tensor_copy(rstdb, rstd)
        mb = psum.tile([P, ntok], f32, tag="big")
        rb = psum.tile([P, ntok], f32, tag="big")
        for t in range(nt):
            sl = slice(t * FD, (t + 1) * FD)
            nc.tensor.matmul(mb[:, sl], ones_blk[0:1, :],
                             meanb[:, sl], start=True, stop=True)
            nc.tensor.matmul(rb[:, sl], ones_blk[0:1, :],
                             rstdb[:, sl], start=True, stop=True)
        out = out_pool.tile([P, KC, ntok], bf16, tag=tag, bufs=1)
        for kc in range(KC):
            src = get_chunk(kc)
            cen = tmp_pool.tile([P, ntok], f32, tag="cen", bufs=1)
            nc.vector.tensor_tensor(cen, src, mb, OP.subtract)
            nc.vector.tensor_tensor(out[:, kc, :], cen, rb, OP.mult)
        return out

    # ---------------- generic feature-major GEMM ----------------
    def gemm_fm(w_of_m, xn, m_chunks, evict, kcs=KC):
        """psum[m,t] = sum_kc w(m)[:, kc, :].T @ xn[:, kc, t*FD:]; evict."""
        for m in range(m_chunks):
            wt, co = w_of_m(m)
            for t in range(NQT):
                sl = slice(t * FD, (t + 1) * FD)
                pt = psum.tile([P, FD], f32, tag="mm")
                for kc in range(kcs):
                    nc.tensor.matmul(pt, wt[:, kc, co:co + P],
                                     xn[:, kc, sl],
                                     start=(kc == 0), stop=(kc == kcs - 1))
                evict(m, t, pt, sl)

    # ---------------- attention (self + cross) ----------------
    def attention(qfm, kfm, v65, o_fm, mask_t):
        def epilogue(h, po):
            # Normalize head h by 1/Z. Z sits in po row 64; move it to
            # partition 0 (reciprocal_approx_fast only works at base 0),
            # evict O to SBUF (frees PSUM early), broadcast 1/Z via a K=1
            # bf16 matmul, scale on DVE (single-PSUM-operand rule).
            kc_h, off = h // 2, (h % 2) * HD
            o65 = probs_pool.tile([P, NQ], bf16, tag="o65", bufs=2)
            zrow = stats.tile([1, NQ], f32, tag="zrow", bufs=1)
            for t in range(NQT):
                sl = slice(t * FD, (t + 1) * FD)
                nc.scalar.activation(o65[0:HD, sl], po[t][0:HD, :], AF.Copy)
                nc.vector.tensor_copy(zrow[0:1, sl], po[t][HD:HD + 1, :])
            zr = stats.tile([1, NQ], f32, tag="zr", bufs=1)
            nc.vector.reciprocal_approx_fast(zr, zrow)
            zrb = stats.tile([1, NQ], bf16, tag="zrb", bufs=1)
            nc.vector.tensor_copy(zrb, zr)
            for t in range(NQT):
                sl = slice(t * FD, (t + 1) * FD)
                zb = psum.tile([P, FD], f32, tag="mm")
                nc.tensor.matmul(zb[0:HD, :], ones_blk[0:1, 0:HD],
                                 zrb[:, sl], start=True, stop=True)
                nc.vector.tensor_tensor(o_fm[off:off + HD, kc_h, sl],
                                        o65[0:HD, sl], zb[0:HD, :], OP.mult)

        prev = None
        for h in range(H):
            kc_h, off = h // 2, (h % 2) * HD
            po = [psum.tile([P, FD], f32, tag="mm", name=f"po{_t}")
                  for _t in range(NQT)]
            for nkc in range(NKC):
                ps = psum.tile([P, NQ], f32, tag="big")
                for t in range(NQT):
                    sl = slice(t * FD, (t + 1) * FD)
                    nc.tensor.matmul(
                        ps[:, sl],
                        kfm[off:off + HD, kc_h, nkc * P:(nkc + 1) * P],
                        qfm[off:off + HD, kc_h, sl],
                        start=True, stop=True)
                pr = probs_pool.tile([P, NQ], bf16, tag="probs", bufs=3)
                nc.scalar.activation(pr, ps, AF.Exp, scale=SCALE)
                if mask_t is not None:
                    nc.gpsimd.tensor_tensor(pr, pr, mask_t[:, nkc, :], OP.mult)
                for t in range(NQT):
                    sl = slice(t * FD, (t + 1) * FD)
                    nc.tensor.matmul(po[t][0:HD + 1, :],
                                     v65[:, nkc, h, :], pr[:, sl],
                                     start=(nkc == 0), stop=(nkc == NKC - 1))
            if prev is not None:
                epilogue(*prev)
            prev = (h, po)
        epilogue(*prev)

    def v_gemm(xn_stationary, wv_dram, bias_t, attn_pool):
        """Token-major V with appended ones col: [128, NKC, H, HD+1] bf16."""
        wv_t = stream_w(chunked(wv_dram))
        v65 = attn_pool.tile([P, NKC, H, HD + 1], bf16, tag="v65", bufs=1)
        bbs = None
        if bias_t is not None:
            bias_b = const.tile([1, C], bf16, name="bias_b", tag="c_biasb")
            nc.vector.tensor_copy(bias_b, bias_t)
            bbp = psum.tile([P, NQ], f32, tag="big")
            nc.tensor.matmul(bbp[:, 0:FD], ones_blk[0:1, :],
                             bias_b[:, 0:FD], start=True, stop=True)
            nc.tensor.matmul(bbp[:, FD:C], ones_blk[0:1, :],
                             bias_b[:, FD:C], start=True, stop=True)
            bbs = const.tile([P, C], f32, name="bbs", tag="c_bbs")
            nc.scalar.activation(bbs, bbp[:, 0:C], AF.Copy)
        for nkc in range(NKC):
            pv = psum.tile([P, NQ], f32, tag="big")
            for kc in range(KC):
                nc.tensor.matmul(pv[:, 0:FD],
                                 xn_stationary[:, kc, nkc * P:(nkc + 1) * P],
                                 wv_t[:, kc, 0:FD],
                                 start=(kc == 0), stop=(kc == KC - 1))
                nc.tensor.matmul(pv[:, FD:C],
                                 xn_stationary[:, kc, nkc * P:(nkc + 1) * P],
                                 wv_t[:, kc, FD:C],
                                 start=(kc == 0), stop=(kc == KC - 1))
            nc.vector.memset(v65[:, nkc, :, HD:HD + 1], 1.0)
            dst = v65[:, nkc, :, 0:HD]
            pv_h = pv[:, 0:C].rearrange("p (h d) -> p h d", d=HD)
            if bbs is not None:
                nc.vector.tensor_tensor(
                    dst, pv_h, bbs.rearrange("p (h d) -> p h d", d=HD), OP.add)
            else:
                nc.scalar.activation(dst, pv_h, AF.Copy)
        return v65

    # ================= program =================
    x0 = masters.tile([P, KC, NQ], f32, tag="xmaster")
    nc.sync.dma_start(out=x0, in_=chunked(xT))
    xT_y = chunked(yT)

    with tc.tile_pool(name="attn", bufs=1) as attn_pool:
        # ---- phase A: LN1 + LNy + QKV ----
        with tc.tile_pool(name="phA", bufs=2) as pa:
            xn1 = layernorm(lambda kc: x0[:, kc, :], pa, pa, tag="xn")

            qfm = attn_pool.tile([P, KC, NQ], bf16, tag="qfm", bufs=1)
            kfm = attn_pool.tile([P, KC, NQ], bf16, tag="kfm", bufs=1)
            wq_half = stream_w(chunked(wqkT)[:, :, 0:C])
            wk_half = stream_w(chunked(wqkT)[:, :, C:2 * C])

            def qk_evict(m, t, pt, sl):
                dst = qfm if m < KC else kfm
                nc.scalar.activation(dst[:, m % KC, sl], pt, AF.Identity,
                                     bias=bqk_t[:, m:m + 1])
            gemm_fm(lambda m: (wq_half, m * P) if m < KC
                    else (wk_half, (m - KC) * P), xn1, MQK, qk_evict)

            v65_s = v_gemm(xn1, wvT, bv_t, attn_pool)

            # LNy emitted last in phase A: its stats/normalize chain overlaps
            # the QKV GEMM matmuls above.
            def y_chunk(kc):
                t = pa.tile([P, NK], f32, tag="ystr", bufs=2)
                nc.sync.dma_start(out=t, in_=xT_y[:, kc, :])
                return t
            yn = layernorm(y_chunk, pa, attn_pool, ntok=NK, tag="yn")

        # ---- phase B: self-attention + proj ----
        with tc.tile_pool(name="phB", bufs=2) as pb:
            o_fm = attn_pool.tile([P, KC, NQ], bf16, tag="ofm", bufs=1)
            attention(qfm, kfm, v65_s, o_fm, None)

            x1 = masters.tile([P, KC, NQ], f32, tag="xmaster")
            wproj_t = stream_w(chunked(wprojT))

            def proj_evict(m, t, pt, sl):
                nc.vector.scalar_tensor_tensor(
                    x1[:, m, sl], pt, bproj_t[:, m:m + 1],
                    x0[:, m, sl], OP.add, OP.add)
            gemm_fm(lambda m: (wproj_t, m * P), o_fm, KC, proj_evict)

        # ---- phase C: cross-attention + proj ----
        with tc.tile_pool(name="phC", bufs=2) as pc:
            # K2/V2 depend only on yn (ready since phase A) — emit them first
            # so the PE has dense work while the LN2 stats chain runs.
            k2 = attn_pool.tile([P, KC, NK], bf16, tag="kfm", bufs=1)
            wk2_t = stream_w(chunked(wk2T))

            def k2_evict(m, t, pt, sl):
                nc.scalar.activation(k2[:, m, sl], pt, AF.Identity,
                                     bias=bk2_t[:, m:m + 1])
            gemm_fm(lambda m: (wk2_t, m * P), yn, KC, k2_evict)

            v65_c = v_gemm(yn, wv2T, bv2_t, attn_pool)

            xn2 = layernorm(lambda kc: x1[:, kc, :], pc, pc, tag="xn")

            mask_t = pc.tile([P, NKC, NQ], bf16, tag="mask", bufs=1)
            nc.sync.dma_start(out=mask_t,
                              in_=maskT.rearrange("(kc p) n -> p kc n", p=P))

            q2 = attn_pool.tile([P, KC, NQ], bf16, tag="qfm", bufs=1)
            wq2_t = stream_w(chunked(wq2T))

            def q2_evict(m, t, pt, sl):
                nc.scalar.activation(q2[:, m, sl], pt, AF.Identity,
                                     bias=bq2_t[:, m:m + 1])
            gemm_fm(lambda m: (wq2_t, m * P), xn2, KC, q2_evict)

            o2_fm = attn_pool.tile([P, KC, NQ], bf16, tag="ofm", bufs=1)
            attention(q2, k2, v65_c, o2_fm, mask_t)

            x2 = masters.tile([P, KC, NQ], f32, tag="xmaster")
            wca_t = stream_w(chunked(wcaT))

            def ca_evict(m, t, pt, sl):
                nc.vector.scalar_tensor_tensor(
                    x2[:, m, sl], pt, bca_t[:, m:m + 1],
                    x1[:, m, sl], OP.add, OP.add)
            gemm_fm(lambda m: (wca_t, m * P), o2_fm, KC, ca_evict)

    # ---- phase D: MLP (attn pool closed; h1 reuses its space) ----
    with tc.tile_pool(name="phD", bufs=2) as pd:
        xn3 = layernorm(lambda kc: x2[:, kc, :], pd, pd, tag="xn")

        h1 = pd.tile([P, MH, NQ], bf16, tag="h1", bufs=1)
        w1view = chunked(wfc1T)
        for quarter in range(4):
            w1q = stream_w(w1view[:, :, quarter * KC * P:(quarter + 1) * KC * P])
            for mi in range(KC):
                m = quarter * KC + mi
                for t in range(NQT):
                    sl = slice(t * FD, (t + 1) * FD)
                    pt = psum.tile([P, FD], f32, tag="mm")
                    for kc in range(KC):
                        nc.tensor.matmul(pt, w1q[:, kc, mi * P:(mi + 1) * P],
                                         xn3[:, kc, sl],
                                         start=(kc == 0), stop=(kc == KC - 1))
                    nc.scalar.activation(h1[:, m, sl], pt, AF.Gelu,
                                         bias=bfc1_t[:, m:m + 1])

        xout = masters.tile([P, KC, NQ], f32, tag="xmaster")
        xout_dram = xoutT.rearrange("(kc p) m -> p kc m", p=P)
        w2view = chunked(wfc2T)  # [128, 24, 768]
        for quarter in range(4):
            w2q = stream_w(w2view[:, quarter * KC:(quarter + 1) * KC, :])
            for m in range(KC):
                for t in range(NQT):
                    sl = slice(t * FD, (t + 1) * FD)
                    pt = psum.tile([P, FD], f32, tag="mm")
                    for kq in range(KC):
                        nc.tensor.matmul(pt, w2q[:, kq, m * P:(m + 1) * P],
                                         h1[:, quarter * KC + kq, sl],
                                         start=(kq == 0), stop=(kq == KC - 1))
                    if quarter == 0:
                        nc.vector.scalar_tensor_tensor(
                            xout[:, m, sl], pt, bfc2_t[:, m:m + 1],
                            x2[:, m, sl], OP.add, OP.add)
                    else:
                        nc.vector.tensor_tensor(
                            xout[:, m, sl], pt, xout[:, m, sl], OP.add)
        for m in range(KC):
            nc.sync.dma_start(out=xout_dram[:, m, :], in_=xout[:, m, :])


def _build(flags):
    import concourse.bacc as bacc
    import concourse.tile as tile
    from contextlib import ExitStack

    nc = bacc.Bacc("TRN2", target_bir_lowering=False, debug=False)
    with tile.TileContext(nc) as tc, ExitStack() as ctx:
        _emit(ctx, tc, nc, flags)
    nc.compile()
    return nc


def kernel(**inputs):
    from concourse.bass_utils import run_bass_kernel_spmd

    shared, per_core, flags = _prep(inputs)
    key = tuple(sorted(flags.items()))
    if key not in _CACHE:
        _CACHE[key] = _build(flags)
    nc = _CACHE[key]

    in_maps = []
    for b in range(B):
        m = dict(shared)
        m.update(per_core[b])
        in_maps.append(m)
    res = run_bass_kernel_spmd(nc, in_maps, core_ids=list(range(B)))
    x_out = np.stack([np.ascontiguousarray(np.asarray(r["xoutT"]).T)
                      for r in res.results]).astype(np.float32)
    y_out = np.asarray(inputs["y"], np.float32)
    return (x_out, y_out)
